# revision 58
# baseline (speedup 1.0000x reference)
"""DualSlidingWindowAttention Trainium2 kernel.

Sharding: 8 cores = 2 batches x 4 head-groups. Core (b, m) owns batch b,
q-heads 8m..8m+7, kv-heads 2m, 2m+1. Host sums the 4 partial o-proj outputs
per batch.

Per-core device program (single TileContext, engines overlap aggressively):
  Phase A: kv projections, paced by the xt DMA stream (sync queue) while
           weights stream on the scalar queue. v is PE-transposed to
           [token, D] layout for the o matmuls.
  Phase B: q projection (256 MMs, 8 PSUM banks, weights stationary per
           chunk), producing the interleaved qT layout directly.
  Phase C: block-sparse attention. Per (kv-group, 128-query tile) up to 5
           128-wide kv chunks matter. Scores are computed transposed
           [kv, q] with the 4 heads of the group interleaved in the free
           dim (N=512). Softmax: exp(s/8) on ACT, mask*exp(alibi) multiply
           on GPSIMD, softmax sums via a ones-column appended to v (free on
           the PE). Normalization: reciprocal on DVE, then a K=2 selector
           matmul broadcasts the per-(head,token) reciprocals across the 64
           D partitions; one DVE multiply per unit applies it. The o-proj
           for the first half is interleaved into the tail units to keep
           the PE dense (HAM stays warm); output is stored f16.

All matmul operands are fp16 (FWL weight loads, half-sized DMA);
accumulation is fp32 in PSUM; sums/reciprocals stay fp32.
"""

import sys

sys.path.insert(0, "/opt/trn_rl_repo")

import numpy as np
import concourse.bass as bass
import concourse.bacc as bacc
import concourse.mybir as mybir
import concourse.tile as tile

F32 = mybir.dt.float32
F16 = mybir.dt.float16

HID, H, HK, G, D, T = 2048, 32, 8, 4, 64, 1024
W_ATT, W_SSM = 256, 64
NQT = T // 128  # 8 query tiles
KVG = 2         # kv heads (= head groups) per core
HL = 4          # q heads per kv group

# slot order: [attn_left, ssm_left, attn_full, attn_causal, ssm_causal]
SLOT_SRC = [1, 0, 1, 1, 0]       # 1 = hidden (attn window), 0 = ssm
SLOT_CHOFF = [-2, -1, -1, 0, 0]  # kv chunk offset relative to qtile
SLOT_OFF = [-256, -128, -128, 0, 0]
SLOT_WIN = [W_ATT, W_SSM, W_ATT, W_ATT, W_SSM]


def first_slot(qt):
    return {0: 3, 1: 1}.get(qt, 0)


def build_program():
    nc = bacc.Bacc("TRN2", target_bir_lowering=False, debug=False)

    xt_ssm = nc.declare_dram_parameter("xt_ssm", [HID, T], F16, isOutput=False)
    xt_hid = nc.declare_dram_parameter("xt_hid", [HID, T], F16, isOutput=False)
    wq = nc.declare_dram_parameter("wq", [128, 4, 32, 128], F16, isOutput=False)
    wk = nc.declare_dram_parameter("wk", [128, 16, 128], F16, isOutput=False)
    wv = nc.declare_dram_parameter("wv", [128, 16, 128], F16, isOutput=False)
    wsk = nc.declare_dram_parameter("wsk", [128, 16, 128], F16, isOutput=False)
    wsv = nc.declare_dram_parameter("wsv", [128, 16, 128], F16, isOutput=False)
    wo = nc.declare_dram_parameter("wo", [128, 4, 2048], F16, isOutput=False)
    mconc = nc.declare_dram_parameter("mconc", [128, 10, 512], F16, isOutput=False)
    ident = nc.declare_dram_parameter("ident", [128, 128], F16, isOutput=False)
    sel = nc.declare_dram_parameter("sel", [2, 128], F32, isOutput=False)
    out_t = nc.declare_dram_parameter("out_t", [HID, T], F16, isOutput=True)

    mm = nc.tensor.matmul

    with tile.TileContext(nc) as tc:
        with tc.tile_pool(name="persist", bufs=1) as pers:
            # ---- persistent sbuf tiles (live for the whole kernel) ----
            xt_sb = {}
            for src in range(2):
                for kc in range(16):
                    xt_sb[(src, kc)] = pers.tile(
                        [128, T], F16, tag=f"xt{src}_{kc}", name=f"xt{src}_{kc}")
            qT_sb = pers.tile([128, NQT, HL * 128], F16, tag="qT")
            kT_sb = [pers.tile([128, T], F16, tag=f"kT{s}", name=f"kT{s}")
                     for s in range(2)]
            # v_sb[src][kvh]: [tok-in-chunk, chunk, D+1]; col 64 = ones
            v_sb = [
                [pers.tile([128, NQT, 65], F16, tag=f"v{s}{h}", name=f"v{s}{h}")
                 for h in range(2)]
                for s in range(2)
            ]
            ident_sb = pers.tile([128, 128], F16, tag="ident")
            sel_sb = pers.tile([2, 128], F32, tag="sel")
            oT_sb = pers.tile([128, 4, T], F32, tag="oT")
            oTb_sb = pers.tile([128, 4, T], F16, tag="oTb")
            m_sb = pers.tile([128, 10, 512], F16, tag="mconc")
            wo_sb = pers.tile([128, 4, 2048], F16, tag="wo")

            # ---------------- Phases A+B: projections ----------------
            with tc.tile_pool(name="wqp", bufs=1) as wqp:
                w4_t = {"wsk": wsk, "wsv": wsv, "wk": wk, "wv": wv}
                w4_sb = {}
                for name in ("wsk", "wsv", "wk", "wv"):
                    w4_sb[name] = wqp.tile([128, 16, 128], F16, tag=name, name=name)
                wq_sb = [wqp.tile([128, 32, 128], F16, tag=f"wq{c}", name=f"wq{c}")
                         for c in range(4)]
                stage_sb = [wqp.tile([128, T], F16, tag=f"stg{s}", name=f"stg{s}")
                            for s in range(2)]

                # Small weights on the scalar queue; the fat transfers all
                # on the sync queue ORDERED xt -> wq -> wo, so the kv phase
                # (paced by xt) gets the full HBM bandwidth and wq tiles
                # land just in time for the c-major q loop.
                for name in ("wsk", "wsv", "wk", "wv"):
                    nc.scalar.dma_start(out=w4_sb[name], in_=w4_t[name][:, :, :])
                nc.scalar.dma_start(out=ident_sb, in_=ident[:, :])
                nc.scalar.dma_start(out=sel_sb, in_=sel[:, :])
                for vsrc in range(2):
                    for vh in range(2):
                        nc.vector.memset(v_sb[vsrc][vh][:, :, 64:65], 1.0)
                nc.scalar.dma_start(out=m_sb, in_=mconc[:, :, :])

                for src, xt_t in ((0, xt_ssm), (1, xt_hid)):
                    for kc in range(16):
                        nc.sync.dma_start(
                            out=xt_sb[(src, kc)],
                            in_=xt_t[kc * 128:(kc + 1) * 128, :])
                for c in range(4):
                    nc.sync.dma_start(out=wq_sb[c][:, :, :], in_=wq[:, c, :, :])
                nc.sync.dma_start(out=wo_sb, in_=wo[:, :, :])

                # -- Phase A: kv projections, 4 psum groups per src --
                with (
                    tc.tile_pool(name="kvp", bufs=3, space="PSUM") as kvp,
                    tc.tile_pool(name="tp", bufs=2, space="PSUM") as tp,
                ):
                    for src in range(2):
                        wk_t = w4_sb["wk" if src else "wsk"]
                        wv_t = w4_sb["wv" if src else "wsv"]
                        kps = [kvp.tile([128, 512], F32, tag="kps",
                                        name=f"kps{src}_{h}") for h in range(2)]
                        vps = [kvp.tile([128, 512], F32, tag="vps",
                                        name=f"vps{src}_{h}") for h in range(2)]
                        for kc in range(16):
                            xtile = xt_sb[(src, kc)]
                            for h in range(2):
                                mm(kps[h][:, :], lhsT=wk_t[:, kc, :],
                                   rhs=xtile[:, h * 512:(h + 1) * 512],
                                   start=(kc == 0), stop=(kc == 15))
                                mm(vps[h][:, :], lhsT=wv_t[:, kc, :],
                                   rhs=xtile[:, h * 512:(h + 1) * 512],
                                   start=(kc == 0), stop=(kc == 15))
                        for h in range(2):
                            nc.vector.tensor_copy(
                                kT_sb[src][:, h * 512:(h + 1) * 512],
                                kps[h][:, :])
                            nc.vector.tensor_copy(
                                stage_sb[src][:, h * 512:(h + 1) * 512],
                                vps[h][:, :])
                    # transposes after both srcs' matmuls so the PE never
                    # waits on the DVE stage evacuations mid-stream.
                    for src in range(2):
                        for h in range(2):
                            for j in range(8):
                                tp_t = tp.tile([128, 64], F16, tag="tp")
                                nc.tensor.transpose(
                                    tp_t[:, :],
                                    stage_sb[src][h * 64:(h + 1) * 64,
                                                  j * 128:(j + 1) * 128],
                                    ident_sb[h * 64:(h + 1) * 64,
                                             h * 64:(h + 1) * 64])
                                nc.scalar.copy(v_sb[src][h][:, j, 0:64],
                                               tp_t[:, :])

                # -- Phase B: q projection, all 8 psum banks --
                # c-major so each col-tile's weights are consumed as they
                # stream in, and each c's psums evacuate while the next c
                # accumulates (no evacuation stall at the B->C transition).
                with tc.tile_pool(name="qp", bufs=8, space="PSUM") as qp:
                    for c in range(4):
                        qps = [qp.tile([128, 512], F32, tag="qps",
                                       name=f"qps{c}_{h}") for h in range(2)]
                        for src in range(2):
                            for kc in range(16):
                                xtile = xt_sb[(src, kc)]
                                for h in range(2):
                                    mm(qps[h][:, :],
                                       lhsT=wq_sb[c][:, src * 16 + kc, :],
                                       rhs=xtile[:, h * 512:(h + 1) * 512],
                                       start=(src == 0 and kc == 0),
                                       stop=(src == 1 and kc == 15))
                        # host permutes Wq cols so col-tile c =
                        # [head c (kvg0), head 4+c (kvg1)].
                        for h in range(2):
                            nc.vector.tensor_copy(
                                qT_sb[:, h * 4:(h + 1) * 4,
                                      c * 128:(c + 1) * 128],
                                qps[h][:, :].rearrange(
                                    "p (qt j) -> p qt j", j=128))

            # ---------------- Phase C: attention ----------------
            # qt 4-7 first: 5-slot units keep the PE dense right at phase-C
            # entry (HAM stays warm); the small qt0/qt1 units land where the
            # first oproj half interleaves.
            QT_ORDER = [4, 5, 6, 7, 0, 1, 2, 3]
            units = [(kvg, qt) for qt in QT_ORDER for kvg in range(KVG)]

            with (
                tc.tile_pool(name="attn_sb", bufs=1) as asb,
                tc.tile_pool(name="weip", bufs=4) as weip,
                tc.tile_pool(name="ostgp", bufs=2) as ostgp,
                tc.tile_pool(name="outstgp", bufs=4) as outstgp,
            ):
                # Softmax-sum staging: sums_st[j, par*16 + t*8 + uu] keeps
                # the reciprocal 128-partition-parallel. rsum
                # [par, j*16 + t*8 + uu] is the K=2 bcast-matmul rhs.
                # Column blocks are reused across halves.
                sums_st = asb.tile([128, 32], F32, tag="sums_st")
                rst = asb.tile([128, 32], F32, tag="rst")
                rsum = asb.tile([2, 128 * 16], F32, tag="rsum")
                wei_tiles = {}

                def emit_scores(u):
                    kvg, qt = units[u]
                    fs = first_slot(qt)
                    wei_t = weip.tile([128, 5, 512], F16, tag="wei")
                    wei_tiles[u] = wei_t
                    for s in range(fs, 5):
                        ch = qt + SLOT_CHOFF[s]
                        sp_t = sp.tile([128, 512], F32, tag="sp")
                        mm(sp_t[:, :],
                           lhsT=kT_sb[SLOT_SRC[s]][kvg * 64:(kvg + 1) * 64,
                                                   ch * 128:(ch + 1) * 128],
                           rhs=qT_sb[kvg * 64:(kvg + 1) * 64, qt, :],
                           start=True, stop=True)
                        nc.scalar.activation(
                            out=wei_t[:, s, :], in_=sp_t[:, :],
                            func=mybir.ActivationFunctionType.Exp, scale=0.125)
                    # DVE only: GPSIMD takes ~4.5us for this multiply and
                    # stalls the in-order PE stream at the o matmuls.
                    nc.vector.tensor_mul(
                        wei_t[:, fs:5, :], wei_t[:, fs:5, :],
                        m_sb[:, kvg * 5 + fs:kvg * 5 + 5, :])

                def emit_o(u):
                    kvg, qt = units[u]
                    fs = first_slot(qt)
                    wei_t = wei_tiles.pop(u)
                    op_t = op.tile([128, 512], F32, tag="op")
                    for s in range(fs, 5):
                        ch = qt + SLOT_CHOFF[s]
                        mm(op_t[0:65, :],
                           lhsT=v_sb[SLOT_SRC[s]][kvg][:, ch, :],
                           rhs=wei_t[:, s, :],
                           start=(s == fs), stop=(s == 4))
                    # scatter unnormalized o into oT and the softmax-sum row
                    # into the recip staging layout. Engines cannot cross
                    # partitions, so stage in SBUF and scatter with DMAs.
                    ostg = ostgp.tile([128, 512], F32, tag="ostg")
                    nc.vector.tensor_copy(ostg[0:65, :], op_t[0:65, :])
                    src4 = ostg[:, :].rearrange("p (t pr j) -> p t pr j",
                                                t=2, pr=2)
                    uu = u % 8
                    for par in range(2):
                        nc.sync.dma_start(
                            out=oT_sb[par * 64:(par + 1) * 64,
                                      kvg * 2:kvg * 2 + 2,
                                      qt * 128:(qt + 1) * 128],
                            in_=src4[0:64, :, par, :])
                    # sums row 64 -> one [1,128]->[128,1] DMA per (par, t),
                    # split across the sync and gpsimd queues.
                    for par in range(2):
                        for t in range(2):
                            c = par * 16 + t * 8 + uu
                            eng = nc.sync if t == 0 else nc.gpsimd
                            eng.dma_start(
                                out=sums_st[:, c:c + 1],
                                in_=src4[64:65, t, par, :])

                def emit_norm_units(u0, nu):
                    # normalize units u0..u0+nu-1 (uu = u0%8 .. +nu-1). The
                    # reciprocal covers all 32 staged columns (cheap); only
                    # the fresh uu-range is shipped to rsum and consumed.
                    uu0 = u0 % 8
                    nc.vector.reciprocal(rst[:, :], sums_st[:, :])
                    for par in range(2):
                        nc.sync.dma_start(
                            out=rsum[par:par + 1, :].rearrange(
                                "p (j t u) -> p j t u",
                                t=2, u=8)[:, :, :, uu0:uu0 + nu],
                            in_=rst[:, par * 16:(par + 1) * 16].rearrange(
                                "p (t u) -> p t u", u=8)[:, :, uu0:uu0 + nu])
                    rs4 = rsum[0:2, :].rearrange("p (j t u) -> p u t j",
                                                 t=2, u=8)
                    for i in range(uu0, uu0 + nu):
                        u = (u0 // 8) * 8 + i
                        kvg, qt = units[u]
                        bc_t = misc.tile([128, 512], F32, tag="misc")
                        mm(bc_t[:, 0:256], lhsT=sel_sb[:, :],
                           rhs=rs4[:, i, :, :], start=True, stop=True)
                        nc.vector.tensor_mul(
                            oTb_sb[:, kvg * 2:kvg * 2 + 2,
                                   qt * 128:(qt + 1) * 128],
                            oT_sb[:, kvg * 2:kvg * 2 + 2,
                                  qt * 128:(qt + 1) * 128],
                            bc_t[:, 0:256].rearrange("p (t j) -> p t j", j=128))

                def emit_oproj(hh, ns, pool, tag):
                    for n in ns:
                        p3_t = pool.tile([128, 512], F32, tag=tag)
                        for c in range(4):
                            mm(p3_t[:, :],
                               lhsT=wo_sb[:, c, n * 128:(n + 1) * 128],
                               rhs=oTb_sb[:, c, hh * 512:(hh + 1) * 512],
                               start=(c == 0), stop=(c == 3))
                        outstg = outstgp.tile([128, 512], F16, tag="outstg")
                        if n % 2 == 0:
                            nc.vector.tensor_copy(outstg[:, :], p3_t[:, :])
                        else:
                            nc.scalar.copy(outstg[:, :], p3_t[:, :])
                        nc.sync.dma_start(
                            out=out_t[n * 128:(n + 1) * 128,
                                      hh * 512:(hh + 1) * 512],
                            in_=outstg[:, :])

                # first norm/oproj covers units 0-7 (= qt 4-7 = tokens
                # 512-1023 = hh 1), interleaved into units 11..15
                OPROJ_A = {11: range(0, 3), 12: range(3, 6), 13: range(6, 9),
                           14: range(9, 12), 15: range(12, 16)}
                with (
                    tc.tile_pool(name="sp", bufs=4, space="PSUM") as sp_,
                    tc.tile_pool(name="op", bufs=2, space="PSUM") as op_,
                    tc.tile_pool(name="misc", bufs=2, space="PSUM") as misc_,
                ):
                    sp, op, misc = sp_, op_, misc_
                    for u in range(len(units)):
                        emit_scores(u)
                        if u >= 3:
                            emit_o(u - 3)
                        if u < 10:
                            # warm-keeper: a ~130ns dummy transpose between
                            # unit chains resets the HAM idle window during
                            # the ACT-paced stretch so the PE never throttles.
                            wt = misc.tile([128, 512], F32, tag="misc",
                                           name=f"warm{u}")
                            nc.tensor.transpose(
                                wt[:, 0:64].bitcast(F16)[:, 0:64],
                                kT_sb[0][0:64, 0:128], ident_sb[0:64, 0:64])
                        if u == 10:
                            emit_norm_units(0, 8)
                        if u == 15:
                            # units 8-11 are done (o(12) just ran): normalize
                            # them under the unit stream so the tail only
                            # waits on units 12-15.
                            emit_norm_units(8, 4)
                        if u in OPROJ_A:
                            emit_oproj(1, OPROJ_A[u], misc, "misc")
                    for u in (13, 14, 15):
                        emit_o(u)
                    emit_norm_units(12, 4)
                # tail: the unit-region psum pools are closed; a 4-deep pool
                # lets the last 64 oproj matmuls stream without copy stalls.
                with tc.tile_pool(name="tailp", bufs=4, space="PSUM") as tailp:
                    emit_oproj(0, range(16), tailp, "tailp")

    nc.finalize()
    return nc


def make_mconc(m):
    """Mask*exp(alibi) tile for core head-group m: [128, 10, 512] f16."""
    p = np.arange(128)[:, None]
    j = np.arange(128)[None, :]
    out = np.zeros((128, 10, 512), np.float16)
    for kvg in range(KVG):
        for s in range(5):
            rel = SLOT_OFF[s] + p - j  # [128, 128] kv - q
            mask = (-rel >= 0) & (-rel < SLOT_WIN[s])
            for hl in range(HL):
                hg = 8 * m + kvg * 4 + hl
                slope = 2.0 ** (-8.0 * hg / H)
                vals = np.where(mask, np.exp(slope * rel.astype(np.float64)), 0.0)
                out[:, kvg * 5 + s, hl * 128:(hl + 1) * 128] = vals.astype(np.float16)
    return out


def make_inputs(core, hidden_states, ssm_states, Wq, Wk, Wv, Wsk, Wsv, Wo):
    b, m = core // 4, core % 4
    f16 = lambda x: np.ascontiguousarray(np.asarray(x, dtype=np.float16))

    def wshard(W, cols, nchunk):
        # [K, cols] -> [128, K//128, cols]
        Ws = np.asarray(W)[:, cols]
        return f16(Ws.reshape(nchunk, 128, Ws.shape[1]).transpose(1, 0, 2))

    # col-tile c = [head c (kvg0) cols, head 4+c (kvg1) cols]
    qperm = np.concatenate(
        [np.arange(64) + 64 * h for c in range(4) for h in (c, 4 + c)])
    qcols = 512 * m + qperm
    kvcols = slice(128 * m, 128 * (m + 1))
    wq_sh = wshard(Wq, qcols, 32)                      # [128, 32, 512]
    wq_sh = np.ascontiguousarray(
        wq_sh.reshape(128, 32, 4, 128).transpose(0, 2, 1, 3))  # c-major
    wo_sh = np.asarray(Wo)[512 * m:512 * (m + 1), :]
    sel = np.zeros((2, 128), np.float32)
    sel[0, 0:64] = 1.0
    sel[1, 64:128] = 1.0
    return {
        "xt_ssm": f16(np.asarray(ssm_states)[b].T),
        "xt_hid": f16(np.asarray(hidden_states)[b].T),
        "wq": wq_sh,
        "wk": wshard(Wk, kvcols, 16),
        "wv": wshard(Wv, kvcols, 16),
        "wsk": wshard(Wsk, kvcols, 16),
        "wsv": wshard(Wsv, kvcols, 16),
        "wo": f16(wo_sh.reshape(4, 128, 2048).transpose(1, 0, 2)),
        "mconc": make_mconc(m),
        "ident": np.eye(128, dtype=np.float16),
        "sel": sel,
    }


def gather(results):
    out = np.zeros((2, T, HID), np.float32)
    for core in range(8):
        b = core // 4
        out[b] += results[core]["out_t"].T.astype(np.float32)
    return out


# ----------------------------------------------------------------------------
# Harness entry point
# ----------------------------------------------------------------------------
_NC_CACHE = []


def _get_program():
    if not _NC_CACHE:
        _NC_CACHE.append(build_program())
    return _NC_CACHE[0]


def _run(inp, trace=False):
    from concourse.bass_utils import run_bass_kernel_spmd

    nc = _get_program()
    in_maps = [make_inputs(core, **{k: np.asarray(inp[k]) for k in (
        "hidden_states", "ssm_states", "Wq", "Wk", "Wv", "Wsk", "Wsv", "Wo")})
        for core in range(8)]
    res = run_bass_kernel_spmd(nc, in_maps, list(range(8)), trace=trace)
    return gather(res.results), res.exec_time_ns


def kernel(hidden_states, ssm_states, Wq, Wk, Wv, Wsk, Wsv, Wo):
    out, _ = _run(dict(
        hidden_states=hidden_states, ssm_states=ssm_states, Wq=Wq, Wk=Wk,
        Wv=Wv, Wsk=Wsk, Wsv=Wsv, Wo=Wo))
    return out
